# revision 5
# baseline (speedup 1.0000x reference)
"""Segment mean-pool (LocalPooling1D) Trainium2 Bass kernel.

x [32, 8192, 256] f32, x_pos [32, 65] sorted int32 boundaries -> y [32, 64, 256].
y[b, j] = mean(x[b, x_pos[b,j]:x_pos[b,j+1]]), empty segments -> 0.

Strategy: data-parallel over batch, 4 rows per core on 8 cores. Token t of a
row maps to SBUF partition p = t // 64, free-slot q = t % 64, so every
partition's x data is one contiguous 64 KB HBM chunk -> large DMA descriptors
at near-peak HBM bandwidth. The 0/1 segment-indicator
ind[p, q, j] = (pos[j] <= 64p + q < pos[j+1]) is built on the DVE per x-chunk
(so the first matmul can start a few us in, not after a whole-row build), from
a tiny [128, QTOK] q-iota and a [128, P] broadcast of pos, both double
stride-0-broadcast into the fused compare. Segment sums accumulate on the
TensorEngine as psum += ind_q.T @ x_q over the 64 q-slices, with even/odd q
packed into separate PE column groups (concurrent sub-array matmuls). Finally
y = (psum_even + psum_odd) * 1/max(count, 1).
"""

import os
import sys

import numpy as np

sys.path.insert(0, "/opt/trn_rl_repo")

import concourse.bacc as bacc
import concourse.bass as bass
import concourse.tile as tile
from concourse import mybir
from concourse.bass_utils import run_bass_kernel_spmd

dt = mybir.dt
Alu = mybir.AluOpType

# Problem constants (hardcoded per harness contract).
B, T, C, P = 32, 8192, 256, 65
NSEG = P - 1
NCORES = 8
R = B // NCORES          # batch rows per core
NPART = 128              # SBUF partitions
QTOK = T // NPART        # 64 tokens per partition (contiguous in HBM)

CFG = {
    "chunkq": int(os.environ.get("KB_CHUNKQ", "16")),      # q-slices per x DMA
    "col_pack": os.environ.get("KB_COLPACK", "1") == "1",  # even/odd PE col groups
    "x_bufs": int(os.environ.get("KB_XBUFS", "8")),
    "ind_bufs": int(os.environ.get("KB_INDBUFS", "6")),
    "s_bufs": int(os.environ.get("KB_SBUFS", "3")),
    "psum_bufs": int(os.environ.get("KB_PSUMBUFS", "2")),
    "dual_dma": os.environ.get("KB_DUALDMA", "1") == "1",
}


def build_program(cfg=CFG):
    chunkq = cfg["chunkq"]
    nchunk = QTOK // chunkq
    col_pack = cfg["col_pack"]

    nc = bacc.Bacc("TRN2", target_bir_lowering=False, debug=False)

    x_d = nc.dram_tensor("x", [R, T, C], dt.float32, kind="ExternalInput")
    pos_d = nc.dram_tensor("x_pos", [R, P], dt.int32, kind="ExternalInput")
    y_d = nc.dram_tensor("y", [R, NSEG, C], dt.float32, kind="ExternalOutput")

    with tile.TileContext(nc) as tc:
        with (
            tc.tile_pool(name="const", bufs=1) as constp,
            tc.tile_pool(name="xp", bufs=cfg["x_bufs"]) as xp,
            tc.tile_pool(name="sp", bufs=cfg["s_bufs"]) as sp,
            tc.tile_pool(name="indp", bufs=cfg["ind_bufs"]) as indp,
            tc.tile_pool(name="smallp", bufs=R) as smallp,
            tc.tile_pool(name="outp", bufs=2) as outp,
            tc.tile_pool(name="psp", bufs=cfg["psum_bufs"], space="PSUM") as psp,
        ):
            # q (token index within partition) along the free axis: [128, 64].
            q_sm = constp.tile([NPART, QTOK], dt.float32)
            nc.gpsimd.iota(q_sm[:], pattern=[[1, QTOK]], base=0,
                           channel_multiplier=0, allow_small_or_imprecise_dtypes=True)
            # 64*p as a per-partition scalar (<= 8128, exact in f32).
            p64_iota = constp.tile([NPART, 1], dt.float32)
            nc.gpsimd.iota(p64_iota[:], pattern=[[1, 1]], base=0, channel_multiplier=QTOK,
                           allow_small_or_imprecise_dtypes=True)

            # Critical startup chain first: one SWDGE DMA per row loads pos
            # broadcast to all 128 partitions AND cast int32->f32 (stride-0
            # partition source). Avoids the gpsimd PartitionBroadcast custom
            # op, whose first use stalls ~10us on a Q7 library reload.
            pos_bs = []
            for r in range(R):
                pos_b = smallp.tile([NPART, P], dt.float32, tag="posb")
                nc.gpsimd.dma_start(pos_b[:], pos_d[r : r + 1, :].broadcast_to((NPART, P)))
                pos_bs.append(pos_b)

            # Off the critical path: segment counts -> 1/max(cnt, 1) per row.
            pos_los, pos_his = [], []
            for r in range(R):
                pos_lo = smallp.tile([NSEG, 1], dt.int32, tag="poslo")
                pos_hi = smallp.tile([NSEG, 1], dt.int32, tag="poshi")
                nc.gpsimd.dma_start(pos_lo[:], pos_d[r : r + 1, 0:NSEG].rearrange("one p -> p one"))
                nc.gpsimd.dma_start(pos_hi[:], pos_d[r : r + 1, 1:P].rearrange("one p -> p one"))
                pos_los.append(pos_lo)
                pos_his.append(pos_hi)

            for r in range(R):
                pos_b = pos_bs[r]

                ps = psp.tile([2 * NSEG if col_pack else NSEG, C], dt.float32)
                # Row as [128 partitions, 64*256]: partition p's line is the
                # contiguous HBM range of tokens [64p, 64p+64).
                xr = x_d[r].rearrange("(p q) c -> p (q c)", p=NPART)
                for ci in range(nchunk):
                    # S[p, k, j] = (pos[j] <= 64p + q), q = ci*chunkq + k.
                    S_c = sp.tile([NPART, chunkq, P], dt.float32, tag="sall")
                    nc.vector.scalar_tensor_tensor(
                        S_c[:],
                        pos_b[:, None, :].broadcast_to((NPART, chunkq, P)),
                        p64_iota[:],
                        q_sm[:, ci * chunkq : (ci + 1) * chunkq, None].broadcast_to(
                            (NPART, chunkq, P)
                        ),
                        op0=Alu.subtract,
                        op1=Alu.is_le,
                    )
                    # ind[p, k, j] = S[p, k, j] - S[p, k, j+1]
                    ind_c = indp.tile([NPART, chunkq, NSEG], dt.float32, tag="ind")
                    nc.vector.tensor_tensor(
                        ind_c[:], S_c[:, :, 0:NSEG], S_c[:, :, 1:P], op=Alu.subtract
                    )

                    xt = xp.tile([NPART, chunkq * C], dt.float32)
                    eng = nc.scalar if (cfg["dual_dma"] and ci % 2) else nc.sync
                    eng.dma_start(xt[:], xr[:, ci * chunkq * C : (ci + 1) * chunkq * C])
                    for k in range(chunkq):
                        q = ci * chunkq + k
                        rhs = xt[:, k * C : (k + 1) * C]
                        lhsT = ind_c[:, k, :]
                        if col_pack:
                            half = q % 2
                            nc.tensor.matmul(
                                ps[half * NSEG : (half + 1) * NSEG, :], lhsT, rhs,
                                start=(q == half), stop=(q == QTOK - 2 + half),
                                tile_position=(0, half * NSEG),
                                skip_group_check=True,
                            )
                        else:
                            nc.tensor.matmul(
                                ps[:], lhsT, rhs,
                                start=(q == 0), stop=(q == QTOK - 1),
                            )

                # Segment counts (off matmul critical path, before the scale).
                cnt_f = smallp.tile([NSEG, 1], dt.float32, tag="cnt")
                nc.vector.tensor_tensor(cnt_f[:], pos_his[r][:], pos_los[r][:], op=Alu.subtract)
                cntc = smallp.tile([NSEG, 1], dt.float32, tag="cntc")
                nc.vector.tensor_scalar(cntc[:], cnt_f[:], 1.0, None, op0=Alu.max)
                recip = smallp.tile([NSEG, 1], dt.float32, tag="recip")
                nc.vector.reciprocal(recip[:], cntc[:])

                out_t = outp.tile([NSEG, C], dt.float32)
                if col_pack:
                    # DVE reads one PSUM operand per op: scale each half alone.
                    half_t = outp.tile([NSEG, C], dt.float32, tag="half")
                    nc.vector.tensor_scalar(
                        half_t[:], ps[NSEG : 2 * NSEG, :], recip[:], None, op0=Alu.mult
                    )
                    nc.vector.scalar_tensor_tensor(
                        out_t[:], ps[0:NSEG, :], recip[:], half_t[:],
                        op0=Alu.mult, op1=Alu.add,
                    )
                else:
                    nc.vector.tensor_scalar(out_t[:], ps[:], recip[:], None, op0=Alu.mult)
                nc.gpsimd.dma_start(y_d[r], out_t[:])

    nc.compile()
    return nc


_PROGRAM = None


def _get_program():
    global _PROGRAM
    if _PROGRAM is None:
        _PROGRAM = build_program()
    return _PROGRAM


def kernel(x, x_pos):
    x = np.ascontiguousarray(x, dtype=np.float32)
    x_pos = np.ascontiguousarray(x_pos, dtype=np.int32)
    nc = _get_program()
    in_maps = [
        {"x": x[c * R : (c + 1) * R], "x_pos": x_pos[c * R : (c + 1) * R]}
        for c in range(NCORES)
    ]
    res = run_bass_kernel_spmd(nc, in_maps, list(range(NCORES)))
    y = np.concatenate([res.results[c]["y"] for c in range(NCORES)], axis=0)
    return y.astype(np.float32)


# revision 7
# speedup vs baseline: 1.0024x; 1.0024x over previous
"""Segment mean-pool (LocalPooling1D) Trainium2 Bass kernel.

x [32, 8192, 256] f32, x_pos [32, 65] sorted int32 boundaries -> y [32, 64, 256].
y[b, j] = mean(x[b, x_pos[b,j]:x_pos[b,j+1]]), empty segments -> 0.

Strategy: data-parallel over batch, 4 rows per core on 8 cores. Token t of a
row maps to SBUF partition p = t // 64, free-slot q = t % 64, so every
partition's x data is one contiguous 64 KB HBM chunk -> large DMA descriptors
at near-peak HBM bandwidth. The 0/1 segment-indicator
ind[p, q, j] = (pos[j] <= 64p + q < pos[j+1]) is built on the DVE per x-chunk
(so the first matmul can start a few us in, not after a whole-row build), from
a tiny [128, QTOK] q-iota and a [128, P] broadcast of pos, both double
stride-0-broadcast into the fused compare. Segment sums accumulate on the
TensorEngine as psum += ind_q.T @ x_q over the 64 q-slices, with even/odd q
packed into separate PE column groups (concurrent sub-array matmuls). Finally
y = (psum_even + psum_odd) * 1/max(count, 1).
"""

import os
import sys

import numpy as np

sys.path.insert(0, "/opt/trn_rl_repo")

import concourse.bacc as bacc
import concourse.bass as bass
import concourse.tile as tile
from concourse import mybir
from concourse.bass_utils import run_bass_kernel_spmd

dt = mybir.dt
Alu = mybir.AluOpType

# Problem constants (hardcoded per harness contract).
B, T, C, P = 32, 8192, 256, 65
NSEG = P - 1
NCORES = 8
R = B // NCORES          # batch rows per core
NPART = 128              # SBUF partitions
QTOK = T // NPART        # 64 tokens per partition (contiguous in HBM)

CFG = {
    "chunkq": int(os.environ.get("KB_CHUNKQ", "16")),      # q-slices per x DMA
    "col_pack": os.environ.get("KB_COLPACK", "1") == "1",  # even/odd PE col groups
    "x_bufs": int(os.environ.get("KB_XBUFS", "8")),
    "ind_bufs": int(os.environ.get("KB_INDBUFS", "6")),
    "s_bufs": int(os.environ.get("KB_SBUFS", "3")),
    "psum_bufs": int(os.environ.get("KB_PSUMBUFS", "2")),
    "dual_dma": os.environ.get("KB_DUALDMA", "1") == "1",
}


def build_program(cfg=CFG):
    chunkq = cfg["chunkq"]
    nchunk = QTOK // chunkq
    col_pack = cfg["col_pack"]

    nc = bacc.Bacc("TRN2", target_bir_lowering=False, debug=False)

    x_d = nc.dram_tensor("x", [R, T, C], dt.float32, kind="ExternalInput")
    pos_d = nc.dram_tensor("x_pos", [R, P], dt.int32, kind="ExternalInput")
    y_d = nc.dram_tensor("y", [R, NSEG, C], dt.float32, kind="ExternalOutput")

    with tile.TileContext(nc) as tc:
        with (
            tc.tile_pool(name="const", bufs=1) as constp,
            tc.tile_pool(name="xp", bufs=cfg["x_bufs"]) as xp,
            tc.tile_pool(name="sp", bufs=cfg["s_bufs"]) as sp,
            tc.tile_pool(name="indp", bufs=cfg["ind_bufs"]) as indp,
            tc.tile_pool(name="smallp", bufs=R) as smallp,
            tc.tile_pool(name="outp", bufs=2) as outp,
            tc.tile_pool(name="psp", bufs=cfg["psum_bufs"], space="PSUM") as psp,
            tc.tile_pool(name="pspos", bufs=1, space="PSUM") as pspos,
        ):
            # q (token index within partition) along the free axis: [128, 64].
            q_sm = constp.tile([NPART, QTOK], dt.float32)
            nc.gpsimd.iota(q_sm[:], pattern=[[1, QTOK]], base=0,
                           channel_multiplier=0, allow_small_or_imprecise_dtypes=True)
            # 64*p as a per-partition scalar (<= 8128, exact in f32).
            p64_iota = constp.tile([NPART, 1], dt.float32)
            nc.gpsimd.iota(p64_iota[:], pattern=[[1, 1]], base=0, channel_multiplier=QTOK,
                           allow_small_or_imprecise_dtypes=True)

            # Critical startup chain: tiny pos loads on the HWDGE (sync) queue
            # ahead of the x stream, then broadcast to 128 partitions on the
            # (idle) TensorEngine as ones[1,128].T @ pos[1,P]. Avoids both the
            # gpsimd PartitionBroadcast Q7-library stall (~10us) and SWDGE
            # completion latency under a saturated DMA stream.
            ones_row = constp.tile([1, NPART], dt.float32)
            nc.gpsimd.iota(ones_row[:], pattern=[[0, NPART]], base=1,
                           channel_multiplier=0, allow_small_or_imprecise_dtypes=True)
            pos_rows = []
            for r in range(R):
                pos_row = smallp.tile([1, P], dt.int32, tag="posrow")
                nc.sync.dma_start(pos_row[:], pos_d[r : r + 1, :])
                pos_rows.append(pos_row)
            pos_bs = []
            for r in range(R):
                posf_row = smallp.tile([1, P], dt.float32, tag="posf")
                nc.vector.tensor_copy(posf_row[:], pos_rows[r][:])
                ps_pos = pspos.tile([NPART, P], dt.float32)
                nc.tensor.matmul(ps_pos[:], ones_row[:], posf_row[:],
                                 start=True, stop=True)
                pos_b = smallp.tile([NPART, P], dt.float32, tag="posb")
                nc.vector.tensor_copy(pos_b[:], ps_pos[:])
                pos_bs.append(pos_b)

            # Off the critical path: segment counts -> 1/max(cnt, 1) per row.
            pos_los, pos_his = [], []
            for r in range(R):
                pos_lo = smallp.tile([NSEG, 1], dt.int32, tag="poslo")
                pos_hi = smallp.tile([NSEG, 1], dt.int32, tag="poshi")
                nc.gpsimd.dma_start(pos_lo[:], pos_d[r : r + 1, 0:NSEG].rearrange("one p -> p one"))
                nc.gpsimd.dma_start(pos_hi[:], pos_d[r : r + 1, 1:P].rearrange("one p -> p one"))
                pos_los.append(pos_lo)
                pos_his.append(pos_hi)

            for r in range(R):
                pos_b = pos_bs[r]

                ps = psp.tile([2 * NSEG if col_pack else NSEG, C], dt.float32)
                # Row as [128 partitions, 64*256]: partition p's line is the
                # contiguous HBM range of tokens [64p, 64p+64).
                xr = x_d[r].rearrange("(p q) c -> p (q c)", p=NPART)
                for ci in range(nchunk):
                    # S[p, k, j] = (pos[j] <= 64p + q), q = ci*chunkq + k.
                    S_c = sp.tile([NPART, chunkq, P], dt.float32, tag="sall")
                    nc.vector.scalar_tensor_tensor(
                        S_c[:],
                        pos_b[:, None, :].broadcast_to((NPART, chunkq, P)),
                        p64_iota[:],
                        q_sm[:, ci * chunkq : (ci + 1) * chunkq, None].broadcast_to(
                            (NPART, chunkq, P)
                        ),
                        op0=Alu.subtract,
                        op1=Alu.is_le,
                    )
                    # ind[p, k, j] = S[p, k, j] - S[p, k, j+1]
                    ind_c = indp.tile([NPART, chunkq, NSEG], dt.float32, tag="ind")
                    nc.vector.tensor_tensor(
                        ind_c[:], S_c[:, :, 0:NSEG], S_c[:, :, 1:P], op=Alu.subtract
                    )

                    xt = xp.tile([NPART, chunkq * C], dt.float32)
                    eng = nc.scalar if (cfg["dual_dma"] and ci % 2) else nc.sync
                    eng.dma_start(xt[:], xr[:, ci * chunkq * C : (ci + 1) * chunkq * C])
                    for k in range(chunkq):
                        q = ci * chunkq + k
                        rhs = xt[:, k * C : (k + 1) * C]
                        lhsT = ind_c[:, k, :]
                        if col_pack:
                            half = q % 2
                            nc.tensor.matmul(
                                ps[half * NSEG : (half + 1) * NSEG, :], lhsT, rhs,
                                start=(q == half), stop=(q == QTOK - 2 + half),
                                tile_position=(0, half * NSEG),
                                skip_group_check=True,
                            )
                        else:
                            nc.tensor.matmul(
                                ps[:], lhsT, rhs,
                                start=(q == 0), stop=(q == QTOK - 1),
                            )

                # Segment counts (off matmul critical path, before the scale).
                cnt_f = smallp.tile([NSEG, 1], dt.float32, tag="cnt")
                nc.vector.tensor_tensor(cnt_f[:], pos_his[r][:], pos_los[r][:], op=Alu.subtract)
                cntc = smallp.tile([NSEG, 1], dt.float32, tag="cntc")
                nc.vector.tensor_scalar(cntc[:], cnt_f[:], 1.0, None, op0=Alu.max)
                recip = smallp.tile([NSEG, 1], dt.float32, tag="recip")
                nc.vector.reciprocal(recip[:], cntc[:])

                out_t = outp.tile([NSEG, C], dt.float32)
                if col_pack:
                    # DVE reads one PSUM operand per op: scale each half alone.
                    half_t = outp.tile([NSEG, C], dt.float32, tag="half")
                    nc.vector.tensor_scalar(
                        half_t[:], ps[NSEG : 2 * NSEG, :], recip[:], None, op0=Alu.mult
                    )
                    nc.vector.scalar_tensor_tensor(
                        out_t[:], ps[0:NSEG, :], recip[:], half_t[:],
                        op0=Alu.mult, op1=Alu.add,
                    )
                else:
                    nc.vector.tensor_scalar(out_t[:], ps[:], recip[:], None, op0=Alu.mult)
                nc.gpsimd.dma_start(y_d[r], out_t[:])

    nc.compile()
    return nc


_PROGRAM = None


def _get_program():
    global _PROGRAM
    if _PROGRAM is None:
        _PROGRAM = build_program()
    return _PROGRAM


def kernel(x, x_pos):
    x = np.ascontiguousarray(x, dtype=np.float32)
    x_pos = np.ascontiguousarray(x_pos, dtype=np.int32)
    nc = _get_program()
    in_maps = [
        {"x": x[c * R : (c + 1) * R], "x_pos": x_pos[c * R : (c + 1) * R]}
        for c in range(NCORES)
    ]
    res = run_bass_kernel_spmd(nc, in_maps, list(range(NCORES)))
    y = np.concatenate([res.results[c]["y"] for c in range(NCORES)], axis=0)
    return y.astype(np.float32)


# revision 10
# speedup vs baseline: 1.0048x; 1.0024x over previous
"""Segment mean-pool (LocalPooling1D) Trainium2 Bass kernel.

x [32, 8192, 256] f32, x_pos [32, 65] sorted int32 boundaries -> y [32, 64, 256].
y[b, j] = mean(x[b, x_pos[b,j]:x_pos[b,j+1]]), empty segments -> 0.

Strategy: data-parallel over batch, 4 rows per core on 8 cores. Token t of a
row maps to SBUF partition p = t // 64, free-slot q = t % 64, so every
partition's x data is one contiguous 64 KB HBM chunk -> large DMA descriptors
at near-peak HBM bandwidth. The 0/1 segment-indicator
ind[p, q, j] = (pos[j] <= 64p + q < pos[j+1]) is built on the DVE per x-chunk
(so the first matmul can start a few us in, not after a whole-row build), from
a tiny [128, QTOK] q-iota and a [128, P] broadcast of pos, both double
stride-0-broadcast into the fused compare. Segment sums accumulate on the
TensorEngine as psum += ind_q.T @ x_q over the 64 q-slices, with even/odd q
packed into separate PE column groups (concurrent sub-array matmuls). Finally
y = (psum_even + psum_odd) * 1/max(count, 1).
"""

import os
import sys

import numpy as np

sys.path.insert(0, "/opt/trn_rl_repo")

import concourse.bacc as bacc
import concourse.bass as bass
import concourse.tile as tile
from concourse import mybir
from concourse.bass_utils import run_bass_kernel_spmd

dt = mybir.dt
Alu = mybir.AluOpType

# Problem constants (hardcoded per harness contract).
B, T, C, P = 32, 8192, 256, 65
NSEG = P - 1
NCORES = 8
R = B // NCORES          # batch rows per core
NPART = 128              # SBUF partitions
QTOK = T // NPART        # 64 tokens per partition (contiguous in HBM)

CFG = {
    "chunkq": int(os.environ.get("KB_CHUNKQ", "16")),      # q-slices per x DMA
    "col_pack": os.environ.get("KB_COLPACK", "0") == "1",  # even/odd PE col groups
    "x_bufs": int(os.environ.get("KB_XBUFS", "8")),
    "ind_bufs": int(os.environ.get("KB_INDBUFS", "6")),
    "s_bufs": int(os.environ.get("KB_SBUFS", "3")),
    "psum_bufs": int(os.environ.get("KB_PSUMBUFS", "2")),
    "dual_dma": os.environ.get("KB_DUALDMA", "1") == "1",
}


def build_program(cfg=CFG):
    chunkq = cfg["chunkq"]
    nchunk = QTOK // chunkq
    col_pack = cfg["col_pack"]

    nc = bacc.Bacc("TRN2", target_bir_lowering=False, debug=False)

    # float32r: same bit layout as f32; enables the 1-cycle/row PE matmul mode
    # (vs 4 for fp32). The BIR verifier requires matmul operand producers to
    # declare f32r output, so x is f32r end-to-end (DMA is then a plain copy).
    x_d = nc.dram_tensor("x", [R, T, C], dt.float32r, kind="ExternalInput")
    pos_d = nc.dram_tensor("x_pos", [R, P], dt.int32, kind="ExternalInput")
    y_d = nc.dram_tensor("y", [R, NSEG, C], dt.float32, kind="ExternalOutput")

    with tile.TileContext(nc) as tc:
        with (
            tc.tile_pool(name="const", bufs=1) as constp,
            tc.tile_pool(name="xp", bufs=cfg["x_bufs"]) as xp,
            tc.tile_pool(name="sp", bufs=cfg["s_bufs"]) as sp,
            tc.tile_pool(name="indp", bufs=cfg["ind_bufs"]) as indp,
            tc.tile_pool(name="smallp", bufs=R) as smallp,
            tc.tile_pool(name="outp", bufs=2) as outp,
            tc.tile_pool(name="psp", bufs=cfg["psum_bufs"], space="PSUM") as psp,
            tc.tile_pool(name="pspos", bufs=1, space="PSUM") as pspos,
        ):
            # q (token index within partition) along the free axis: [128, 64].
            q_sm = constp.tile([NPART, QTOK], dt.float32)
            nc.gpsimd.iota(q_sm[:], pattern=[[1, QTOK]], base=0,
                           channel_multiplier=0, allow_small_or_imprecise_dtypes=True)
            # 64*p as a per-partition scalar (<= 8128, exact in f32).
            p64_iota = constp.tile([NPART, 1], dt.float32)
            nc.gpsimd.iota(p64_iota[:], pattern=[[1, 1]], base=0, channel_multiplier=QTOK,
                           allow_small_or_imprecise_dtypes=True)

            # Critical startup chain: tiny pos loads on the HWDGE (sync) queue
            # ahead of the x stream, then broadcast to 128 partitions on the
            # (idle) TensorEngine as ones[1,128].T @ pos[1,P]. Avoids both the
            # gpsimd PartitionBroadcast Q7-library stall (~10us) and SWDGE
            # completion latency under a saturated DMA stream.
            ones_row = constp.tile([1, NPART], dt.float32)
            nc.gpsimd.iota(ones_row[:], pattern=[[0, NPART]], base=1,
                           channel_multiplier=0, allow_small_or_imprecise_dtypes=True)
            pos_rows = []
            for r in range(R):
                pos_row = smallp.tile([1, P], dt.int32, tag="posrow")
                nc.sync.dma_start(pos_row[:], pos_d[r : r + 1, :])
                pos_rows.append(pos_row)
            pos_bs = []
            for r in range(R):
                posf_row = smallp.tile([1, P], dt.float32, tag="posf")
                nc.vector.tensor_copy(posf_row[:], pos_rows[r][:])
                ps_pos = pspos.tile([NPART, P], dt.float32)
                nc.tensor.matmul(ps_pos[:], ones_row[:], posf_row[:],
                                 start=True, stop=True)
                pos_b = smallp.tile([NPART, P], dt.float32, tag="posb")
                nc.vector.tensor_copy(pos_b[:], ps_pos[:])
                pos_bs.append(pos_b)

            # Off the critical path: segment counts -> 1/max(cnt, 1) per row.
            pos_los, pos_his = [], []
            for r in range(R):
                pos_lo = smallp.tile([NSEG, 1], dt.int32, tag="poslo")
                pos_hi = smallp.tile([NSEG, 1], dt.int32, tag="poshi")
                nc.gpsimd.dma_start(pos_lo[:], pos_d[r : r + 1, 0:NSEG].rearrange("one p -> p one"))
                nc.gpsimd.dma_start(pos_hi[:], pos_d[r : r + 1, 1:P].rearrange("one p -> p one"))
                pos_los.append(pos_lo)
                pos_his.append(pos_hi)

            for r in range(R):
                pos_b = pos_bs[r]

                ps = psp.tile([2 * NSEG if col_pack else NSEG, C], dt.float32)
                # Row as [128 partitions, 64*256]: partition p's line is the
                # contiguous HBM range of tokens [64p, 64p+64).
                xr = x_d[r].rearrange("(p q) c -> p (q c)", p=NPART)
                for ci in range(nchunk):
                    # S[p, k, j] = (pos[j] <= 64p + q), q = ci*chunkq + k.
                    S_c = sp.tile([NPART, chunkq, P], dt.float32, tag="sall")
                    nc.vector.scalar_tensor_tensor(
                        S_c[:],
                        pos_b[:, None, :].broadcast_to((NPART, chunkq, P)),
                        p64_iota[:],
                        q_sm[:, ci * chunkq : (ci + 1) * chunkq, None].broadcast_to(
                            (NPART, chunkq, P)
                        ),
                        op0=Alu.subtract,
                        op1=Alu.is_le,
                    )
                    # ind[p, k, j] = S[p, k, j] - S[p, k, j+1]
                    ind_c = indp.tile([NPART, chunkq, NSEG], dt.float32r, tag="ind")
                    nc.vector.tensor_tensor(
                        ind_c[:], S_c[:, :, 0:NSEG], S_c[:, :, 1:P], op=Alu.subtract
                    )

                    xt = xp.tile([NPART, chunkq * C], dt.float32r)
                    eng = nc.scalar if (cfg["dual_dma"] and ci % 2) else nc.sync
                    eng.dma_start(xt[:], xr[:, ci * chunkq * C : (ci + 1) * chunkq * C])
                    for k in range(chunkq):
                        q = ci * chunkq + k
                        rhs = xt[:, k * C : (k + 1) * C]
                        lhsT = ind_c[:, k, :]
                        if col_pack:
                            half = q % 2
                            nc.tensor.matmul(
                                ps[half * NSEG : (half + 1) * NSEG, :], lhsT, rhs,
                                start=(q == half), stop=(q == QTOK - 2 + half),
                                tile_position=(0, half * NSEG),
                                skip_group_check=True,
                            )
                        else:
                            nc.tensor.matmul(
                                ps[:], lhsT, rhs,
                                start=(q == 0), stop=(q == QTOK - 1),
                            )

                # Segment counts (off matmul critical path, before the scale).
                cnt_f = smallp.tile([NSEG, 1], dt.float32, tag="cnt")
                nc.vector.tensor_tensor(cnt_f[:], pos_his[r][:], pos_los[r][:], op=Alu.subtract)
                cntc = smallp.tile([NSEG, 1], dt.float32, tag="cntc")
                nc.vector.tensor_scalar(cntc[:], cnt_f[:], 1.0, None, op0=Alu.max)
                recip = smallp.tile([NSEG, 1], dt.float32, tag="recip")
                nc.vector.reciprocal(recip[:], cntc[:])

                out_t = outp.tile([NSEG, C], dt.float32)
                if col_pack:
                    # DVE reads one PSUM operand per op: scale each half alone.
                    half_t = outp.tile([NSEG, C], dt.float32, tag="half")
                    nc.vector.tensor_scalar(
                        half_t[:], ps[NSEG : 2 * NSEG, :], recip[:], None, op0=Alu.mult
                    )
                    nc.vector.scalar_tensor_tensor(
                        out_t[:], ps[0:NSEG, :], recip[:], half_t[:],
                        op0=Alu.mult, op1=Alu.add,
                    )
                else:
                    nc.vector.tensor_scalar(out_t[:], ps[:], recip[:], None, op0=Alu.mult)
                nc.gpsimd.dma_start(y_d[r], out_t[:])

    nc.compile()
    return nc


_PROGRAM = None


def _get_program():
    global _PROGRAM
    if _PROGRAM is None:
        _PROGRAM = build_program()
    return _PROGRAM


def kernel(x, x_pos):
    x = np.ascontiguousarray(x, dtype=np.float32)
    x_pos = np.ascontiguousarray(x_pos, dtype=np.int32)
    nc = _get_program()
    in_maps = [
        {"x": x[c * R : (c + 1) * R], "x_pos": x_pos[c * R : (c + 1) * R]}
        for c in range(NCORES)
    ]
    res = run_bass_kernel_spmd(nc, in_maps, list(range(NCORES)))
    y = np.concatenate([res.results[c]["y"] for c in range(NCORES)], axis=0)
    return y.astype(np.float32)


# revision 15
# speedup vs baseline: 1.0842x; 1.0790x over previous
"""Segment mean-pool (LocalPooling1D) Trainium2 Bass kernel.

x [32, 8192, 256] f32, x_pos [32, 65] sorted int32 boundaries -> y [32, 64, 256].
y[b, j] = mean(x[b, x_pos[b,j]:x_pos[b,j+1]]), empty segments -> 0.

Strategy: data-parallel over batch, 4 rows per core on 8 cores. Token t of a
row maps to SBUF partition p = t // 64, free-slot q = t % 64, so every
partition's x data is one contiguous 64 KB HBM chunk -> large DMA descriptors
at near-peak HBM bandwidth. The 0/1 segment-indicator
ind[p, q, j] = (pos[j] <= 64p + q < pos[j+1]) is built on the DVE per x-chunk
(so the first matmul can start a few us in, not after a whole-row build), from
a tiny [128, QTOK] q-iota and a [128, P] broadcast of pos, both double
stride-0-broadcast into the fused compare. Segment sums accumulate on the
TensorEngine as psum += ind_q.T @ x_q over the 64 q-slices, with even/odd q
packed into separate PE column groups (concurrent sub-array matmuls). Finally
y = (psum_even + psum_odd) * 1/max(count, 1).
"""

import os
import sys

import numpy as np

sys.path.insert(0, "/opt/trn_rl_repo")

import concourse.bacc as bacc
import concourse.bass as bass
import concourse.tile as tile
from concourse import mybir
from concourse.bass_utils import run_bass_kernel_spmd

dt = mybir.dt
Alu = mybir.AluOpType

# Problem constants (hardcoded per harness contract).
B, T, C, P = 32, 8192, 256, 65
NSEG = P - 1
NCORES = 8
R = B // NCORES          # batch rows per core
NPART = 128              # SBUF partitions
QTOK = T // NPART        # 64 tokens per partition (contiguous in HBM)

CFG = {
    "chunkq": int(os.environ.get("KB_CHUNKQ", "16")),      # q-slices per x DMA
    "col_pack": os.environ.get("KB_COLPACK", "0") == "1",  # even/odd PE col groups
    "x_bufs": int(os.environ.get("KB_XBUFS", "8")),
    "ind_bufs": int(os.environ.get("KB_INDBUFS", "6")),
    "s_bufs": int(os.environ.get("KB_SBUFS", "3")),
    "psum_bufs": int(os.environ.get("KB_PSUMBUFS", "2")),
    "dual_dma": os.environ.get("KB_DUALDMA", "1") == "1",
}


def build_program(cfg=CFG):
    chunkq = cfg["chunkq"]
    nchunk = QTOK // chunkq
    col_pack = cfg["col_pack"]

    nc = bacc.Bacc("TRN2", target_bir_lowering=False, debug=False)

    # float32r: same bit layout as f32; enables the 1-cycle/row PE matmul mode
    # (vs 4 for fp32). The BIR verifier requires matmul operand producers to
    # declare f32r output, so x is f32r end-to-end (DMA is then a plain copy).
    x_d = nc.dram_tensor("x", [R, T, C], dt.float32r, kind="ExternalInput")
    pos_d = nc.dram_tensor("x_pos", [R, P], dt.int32, kind="ExternalInput")
    y_d = nc.dram_tensor("y", [R, NSEG, C], dt.float32, kind="ExternalOutput")

    with tile.TileContext(nc) as tc:
        with (
            tc.tile_pool(name="const", bufs=1) as constp,
            tc.tile_pool(name="xp", bufs=cfg["x_bufs"]) as xp,
            tc.tile_pool(name="sp", bufs=cfg["s_bufs"]) as sp,
            tc.tile_pool(name="indp", bufs=cfg["ind_bufs"]) as indp,
            tc.tile_pool(name="smallp", bufs=R) as smallp,
            tc.tile_pool(name="outp", bufs=2) as outp,
            tc.tile_pool(name="psp", bufs=cfg["psum_bufs"], space="PSUM") as psp,
            tc.tile_pool(name="pspos", bufs=1, space="PSUM") as pspos,
            tc.tile_pool(name="xtailp", bufs=4) as xtailp,
            tc.tile_pool(name="stailp", bufs=2) as stailp,
            tc.tile_pool(name="indtailp", bufs=4) as indtailp,
        ):
            # q (token index within partition) along the free axis: [128, 64].
            q_sm = constp.tile([NPART, QTOK], dt.float32)
            nc.gpsimd.iota(q_sm[:], pattern=[[1, QTOK]], base=0,
                           channel_multiplier=0, allow_small_or_imprecise_dtypes=True)
            # 64*p as a per-partition scalar (<= 8128, exact in f32).
            p64_iota = constp.tile([NPART, 1], dt.float32)
            nc.gpsimd.iota(p64_iota[:], pattern=[[1, 1]], base=0, channel_multiplier=QTOK,
                           allow_small_or_imprecise_dtypes=True)

            # Critical startup chain: tiny pos loads on the HWDGE (sync) queue
            # ahead of the x stream, then broadcast to 128 partitions on the
            # (idle) TensorEngine as ones[1,128].T @ pos[1,P]. Avoids both the
            # gpsimd PartitionBroadcast Q7-library stall (~10us) and SWDGE
            # completion latency under a saturated DMA stream.
            ones_row = constp.tile([1, NPART], dt.float32)
            nc.gpsimd.iota(ones_row[:], pattern=[[0, NPART]], base=1,
                           channel_multiplier=0, allow_small_or_imprecise_dtypes=True)
            pos_rows = []
            for r in range(R):
                pos_row = smallp.tile([1, P], dt.int32, tag="posrow")
                nc.sync.dma_start(pos_row[:], pos_d[r : r + 1, :])
                pos_rows.append(pos_row)
            pos_bs = []
            for r in range(R):
                posf_row = smallp.tile([1, P], dt.float32, tag="posf")
                nc.vector.tensor_copy(posf_row[:], pos_rows[r][:])
                ps_pos = pspos.tile([NPART, P], dt.float32)
                nc.tensor.matmul(ps_pos[:], ones_row[:], posf_row[:],
                                 start=True, stop=True)
                pos_b = smallp.tile([NPART, P], dt.float32, tag="posb")
                nc.vector.tensor_copy(pos_b[:], ps_pos[:])
                pos_bs.append(pos_b)

            # Off the critical path: segment counts -> 1/max(cnt, 1) per row.
            pos_los, pos_his = [], []
            for r in range(R):
                pos_lo = smallp.tile([NSEG, 1], dt.int32, tag="poslo")
                pos_hi = smallp.tile([NSEG, 1], dt.int32, tag="poshi")
                nc.gpsimd.dma_start(pos_lo[:], pos_d[r : r + 1, 0:NSEG].rearrange("one p -> p one"))
                nc.gpsimd.dma_start(pos_hi[:], pos_d[r : r + 1, 1:P].rearrange("one p -> p one"))
                pos_los.append(pos_lo)
                pos_his.append(pos_hi)

            for r in range(R):
                pos_b = pos_bs[r]

                ps = psp.tile([2 * NSEG if col_pack else NSEG, C], dt.float32)
                # Row as [128 partitions, 64*256]: partition p's line is the
                # contiguous HBM range of tokens [64p, 64p+64).
                xr = x_d[r].rearrange("(p q) c -> p (q c)", p=NPART)
                # The very last chunk gates the kernel tail (DMA-completion
                # latency + matmul burst + scale + store all serialize after
                # it): split it into small pieces so the post-stream burst is
                # short.
                if r == R - 1:
                    tailq = max(4, chunkq // 4)
                    qsteps = [chunkq] * (nchunk - 1) + [tailq] * (chunkq // tailq)
                else:
                    qsteps = [chunkq] * nchunk
                q0 = 0
                for ci, cq in enumerate(qsteps):
                    tail = cq != chunkq
                    # S[p, k, j] = (pos[j] <= 64p + q), q = q0 + k.
                    S_c = (stailp if tail else sp).tile([NPART, cq, P], dt.float32, tag="sall")
                    nc.vector.scalar_tensor_tensor(
                        S_c[:],
                        pos_b[:, None, :].broadcast_to((NPART, cq, P)),
                        p64_iota[:],
                        q_sm[:, q0 : q0 + cq, None].broadcast_to((NPART, cq, P)),
                        op0=Alu.subtract,
                        op1=Alu.is_le,
                    )
                    # ind[p, k, j] = S[p, k, j] - S[p, k, j+1]
                    ind_c = (indtailp if tail else indp).tile([NPART, cq, NSEG], dt.float32r, tag="ind")
                    nc.vector.tensor_tensor(
                        ind_c[:], S_c[:, :, 0:NSEG], S_c[:, :, 1:P], op=Alu.subtract
                    )

                    xt = (xtailp if tail else xp).tile([NPART, cq * C], dt.float32r, tag="x")
                    eng = nc.scalar if (cfg["dual_dma"] and ci % 2) else nc.sync
                    eng.dma_start(xt[:], xr[:, q0 * C : (q0 + cq) * C])
                    for k in range(cq):
                        q = q0 + k
                        rhs = xt[:, k * C : (k + 1) * C]
                        lhsT = ind_c[:, k, :]
                        if col_pack:
                            half = q % 2
                            nc.tensor.matmul(
                                ps[half * NSEG : (half + 1) * NSEG, :], lhsT, rhs,
                                start=(q == half), stop=(q == QTOK - 2 + half),
                                tile_position=(0, half * NSEG),
                                skip_group_check=True,
                            )
                        else:
                            nc.tensor.matmul(
                                ps[:], lhsT, rhs,
                                start=(q == 0), stop=(q == QTOK - 1),
                            )
                    q0 += cq

                # Segment counts (off matmul critical path, before the scale).
                cnt_f = smallp.tile([NSEG, 1], dt.float32, tag="cnt")
                nc.vector.tensor_tensor(cnt_f[:], pos_his[r][:], pos_los[r][:], op=Alu.subtract)
                cntc = smallp.tile([NSEG, 1], dt.float32, tag="cntc")
                nc.vector.tensor_scalar(cntc[:], cnt_f[:], 1.0, None, op0=Alu.max)
                recip = smallp.tile([NSEG, 1], dt.float32, tag="recip")
                nc.vector.reciprocal(recip[:], cntc[:])

                out_t = outp.tile([NSEG, C], dt.float32)
                if col_pack:
                    # DVE reads one PSUM operand per op: scale each half alone.
                    half_t = outp.tile([NSEG, C], dt.float32, tag="half")
                    nc.vector.tensor_scalar(
                        half_t[:], ps[NSEG : 2 * NSEG, :], recip[:], None, op0=Alu.mult
                    )
                    nc.vector.scalar_tensor_tensor(
                        out_t[:], ps[0:NSEG, :], recip[:], half_t[:],
                        op0=Alu.mult, op1=Alu.add,
                    )
                else:
                    nc.vector.tensor_scalar(out_t[:], ps[:], recip[:], None, op0=Alu.mult)
                if r == R - 1:
                    # Last store is on the kernel critical path: HWDGE (lower
                    # completion latency) on the by-then idle scalar queue.
                    nc.scalar.dma_start(y_d[r], out_t[:])
                else:
                    nc.gpsimd.dma_start(y_d[r], out_t[:])

    nc.compile()
    return nc


_PROGRAM = None


def _get_program():
    global _PROGRAM
    if _PROGRAM is None:
        _PROGRAM = build_program()
    return _PROGRAM


def kernel(x, x_pos):
    x = np.ascontiguousarray(x, dtype=np.float32)
    x_pos = np.ascontiguousarray(x_pos, dtype=np.int32)
    nc = _get_program()
    in_maps = [
        {"x": x[c * R : (c + 1) * R], "x_pos": x_pos[c * R : (c + 1) * R]}
        for c in range(NCORES)
    ]
    res = run_bass_kernel_spmd(nc, in_maps, list(range(NCORES)))
    y = np.concatenate([res.results[c]["y"] for c in range(NCORES)], axis=0)
    return y.astype(np.float32)
